# revision 23
# baseline (speedup 1.0000x reference)
"""Trainium2 Bass kernel for InterpBaselineEncoder (histogram binning), v12.

See reference: coarsen 128x128 grid 4x4 -> 1024 cells; scatter-mean U=8192
off-grid points (+ on-grid cell values) into cells via closed-form binning
round_ne(p*127/4 - 0.375); gather cell averages at T targets.

Bin = 32i+j split as hi = 4i + j//8 (128, PSUM partitions) and lo = j%8.
Scatter: psum[hi, (lo,y')] += oh128(hi)[u] * (oh8(lo)[u] * [y,1][u]); the
ones column produces counts.  The on-grid (pooled) cell values land in
row-major cell order s = 8*hi + lo, i.e. ypsb[128, 8, 9] is ALREADY in
(hi, lo) layout, so they are folded in with a single identity matmul
accumulated into the same psum (plus a constant 1.0 column -> counts+1).
Gather: broadcast target hi rows by selector matmul, one-hot on ACT
(relu(1-(x-q)^2) of integer distance), gather avg rows by matmul,
contract the lo one-hot on DVE.

Perf notes: the binning-critical coords ride in a small first DMA and
the other queue is semaphore-held until it lands (concurrent transfers
share DMA engines round-robin and can stretch the critical transfer
3-5x); a dummy local_scatter warms the gpsimd ucode library (1.4us
load) during the input DMAs; w2 is built pair-interleaved [kk,lo,y,2]
so every DVE operand is packed-innermost and the 2x 16-bit mode
engages, while the matmul moving view w2p[:,k//2,:,:,k%2] (stride-2
columns) costs the PE nothing; ra tiles are split gpsimd/DVE to
balance engine finish; the on-grid identity matmul rides mid-stream so
the psum stop isn't gated on it; the lo contraction is an all-bf16
packed add tree and the output rides out as bf16 (host casts back to
f32), halving the final DMA; output halves ride separate queues.

Sharding: 8 cores = 4 batches x 2 target halves; SPMD, per-core inputs.
"""
import sys
import numpy as np

for _p in ("/opt/trn_rl_repo", "/opt/pypackages"):
    if _p not in sys.path:
        sys.path.insert(0, _p)

import ml_dtypes  # noqa: E402
from concourse import bass, bacc, mybir, tile  # noqa: E402
from concourse.bass import _add_dep_helper  # noqa: E402
from concourse.bass_utils import run_bass_kernel_spmd  # noqa: E402

F32 = mybir.dt.float32
BF16 = mybir.dt.bfloat16
I16 = mybir.dt.int16
ALU = mybir.AluOpType
ACTF = mybir.ActivationFunctionType

B, U, T, Y = 4, 8192, 4096, 8
TH = T // 2            # targets per core (2048)
KT = U // 128          # 64 point tiles
NT = TH // 128         # 16 target tiles
HI, LO = 128, 8        # bin split: bin = 32i + j = 8*hi + lo
CH = 32                # point tiles per w2 chunk
NG = NT // 4           # gather groups of 4 tiles

_RA_CALLS = (12, 12, 12, 12)   # gpsimd local_scatter tile counts
_RA_DVE = 16                   # trailing ra tiles built on DVE
KG = KT - _RA_DVE

# closed-form bin constants: centers c_k = (4k+1.5)/127, step 4/127
_INV = 127.0 / 4.0
_OFF0 = float(np.float32(-(1.5 / 127.0) * _INV))
_MAGIC = 12582912.0  # 1.5*2^23: (z+M)-M rounds to nearest-even integer
# (1.5*2^23 keeps z+M in the unit-spacing zone [2^23, 2^24) even for z<0)

NB = KT + NT  # 80: off-grid then target coords, fused binning
_IN_COLS = 2 * NB + KT // 2              # coords + packed bf16 ones
_CE_COLS = 1 + 1 + 8 + KT + 128 + 128    # iotaP niotaP i8row rabase ident i128
# bf16 blob [128, cols]: ycON(1024, (c,w,y) order) | pmat(32) | identB(128)
_CB_COLS = 1024 + 32 + 128


def build_nc():
    nc = bacc.Bacc("TRN2", target_bir_lowering=False, debug=False)

    inF = nc.declare_dram_parameter("inF", [128, _IN_COLS], F32,
                                    isOutput=False)
    constE = nc.declare_dram_parameter("constE", [128, _CE_COLS], F32,
                                       isOutput=False)
    ybfD = nc.declare_dram_parameter("ybf", [128, KT * 9], BF16,
                                     isOutput=False)
    conB = nc.declare_dram_parameter("conB", [128, _CB_COLS], BF16,
                                     isOutput=False)
    selB = nc.declare_dram_parameter("selB", [16, NT * 128], BF16,
                                     isOutput=False)
    out_d = nc.declare_dram_parameter("out", [TH, Y], BF16, isOutput=True)

    with tile.TileContext(nc) as tc:
        with (
            tc.tile_pool(name="work", bufs=1) as wpool,
            tc.tile_pool(name="psS", bufs=1, space="PSUM") as psS,
            tc.tile_pool(name="psP", bufs=1, space="PSUM") as psP,
            tc.tile_pool(name="psB", bufs=3, space="PSUM") as psB,
            tc.tile_pool(name="psR", bufs=1, space="PSUM") as psR,
        ):
            # ---- input DMAs: binning-critical first, per queue ----
            tin = wpool.tile([128, _IN_COLS], F32, tag="tin")
            i_inf = nc.sync.dma_start(tin[:], inF[:])
            cE = wpool.tile([128, _CE_COLS], F32, tag="cE")
            nc.sync.dma_start(cE[:], constE[:])
            # ybf arrives pair-interleaved: [p, kk, y, k2] with k = 2*kk+k2
            t_ybfp = wpool.tile([128, KT // 2, 9, 2], BF16, tag="ybf")
            i_ybf = nc.scalar.dma_start(
                t_ybfp[:],
                ybfD[:].rearrange("p (k y two) -> p k y two", y=9, two=2))
            # hold the scalar queue until the binning-critical inF transfer
            # lands: concurrent transfers share DMA engines round-robin and
            # can stretch inF from 0.6us to 3us
            _add_dep_helper(i_ybf.ins, i_inf.ins, sync=True,
                            reason="inF transfer gets all DMA engines first")
            cB = wpool.tile([128, _CB_COLS], BF16, tag="cB")
            nc.scalar.dma_start(cB[:], conB[:])
            c_selB = wpool.tile([16, NT * 128], BF16, tag="selB")
            nc.scalar.dma_start(c_selB[:], selB[:])

            o = 0
            c_iotaP = cE[:, o:o + 1]; o += 1          # [128,1] p
            c_niotaP = cE[:, o:o + 1]; o += 1         # [128,1] -p
            c_i8row = cE[:, o:o + 8]; o += 8          # rows 0..7
            c_rabase = cE[:, o:o + KT]; o += KT       # 128*(k - call_start)
            c_ident = cE[:, o:o + 128]; o += 128
            c_i128row = cE[:, o:o + 128]; o += 128
            t_ycon = cB[:, 0:1024]
            c_pmat = cB[:, 1024:1056]
            c_identB = cB[:, 1056:1184]
            c_sel = c_selB[:].rearrange("p (n q) -> p n q", q=128)

            t_coord = tin[:, 0:2 * NB]
            t_ones = tin[:, 2 * NB:2 * NB + KT // 2].bitcast(BF16)

            # ---- gpsimd ucode warm-up: tiny dep-free local_scatter so the
            # 1.4us library load runs during the input DMAs ----
            wdat = wpool.tile([128, 2], BF16, tag="wdat")
            nc.vector.memset(wdat[:], 1.0)
            widx = wpool.tile([128, 2], I16, tag="widx")
            nc.vector.memset(widx[:, 0:1], 0)
            nc.vector.memset(widx[:, 1:2], 1)
            gvabf = wpool.tile([32, 32, 9], BF16, tag="gvabf")
            nc.vector.memset(gvabf[:, :, 8:9], 1.0)
            wdst = wpool.tile([128, 2], BF16, tag="wdst")
            nc.gpsimd.local_scatter(wdst[:], wdat[:], widx[:],
                                    channels=128, num_elems=2, num_idxs=2)

            # ---- fused off-grid + target binning (DVE) ----
            z = wpool.tile([128, 2 * NB], F32, tag="binz")
            idx = wpool.tile([128, 2 * NB], F32, tag="bini")
            nc.vector.tensor_scalar(z[:], t_coord, _INV, _OFF0,
                                    ALU.mult, ALU.add)
            nc.vector.tensor_scalar(idx[:], z[:], _MAGIC, _MAGIC,
                                    ALU.add, ALU.subtract)
            iv, jv = idx[:, 0:NB], idx[:, NB:2 * NB]
            t1 = wpool.tile([128, NB], F32, tag="t1")
            jh = wpool.tile([128, NB], F32, tag="jh")
            jh8 = wpool.tile([128, NB], F32, tag="jh8")
            lov = wpool.tile([128, NB], F32, tag="lov")
            i4 = wpool.tile([128, NB], F32, tag="i4")
            hiv = wpool.tile([128, NB], F32, tag="hiv")
            nc.vector.tensor_scalar(t1[:], jv, 0.125, -0.4999,
                                    ALU.mult, ALU.add)
            nc.vector.tensor_scalar(jh[:], t1[:], _MAGIC, _MAGIC,
                                    ALU.add, ALU.subtract)
            nc.vector.tensor_scalar(jh8[:], jh[:], 8.0, None, ALU.mult)
            nc.vector.tensor_tensor(lov[:], jv, jh8[:], ALU.subtract)
            nc.vector.tensor_scalar(i4[:], iv, 4.0, None, ALU.mult)
            nc.vector.tensor_tensor(hiv[:], i4[:], jh[:], ALU.add)
            hio, loo = hiv[:, 0:KT], lov[:, 0:KT]
            hit, lot = hiv[:, KT:NB], lov[:, KT:NB]

            # ra scatter indices for the gpsimd tiles, emitted right after
            # hi so the ucode chain starts as early as possible
            rai = wpool.tile([128, KG], I16, tag="rai")
            nc.vector.tensor_tensor(rai[:], hio[:, 0:KG],
                                    c_rabase[:, 0:KG], ALU.add)

            # ---- gpsimd: ra local_scatter calls ----
            # gpsimd- and DVE-built tiles live in separate SBUF allocations
            # so the two engines' write bursts don't share sub-banks
            ra = wpool.tile([128, KG, HI], BF16, tag="ra")
            rad = wpool.tile([128, _RA_DVE, HI], BF16, tag="rad")
            s = 0
            for ntile in _RA_CALLS:
                nc.gpsimd.local_scatter(
                    ra[:, s:s + ntile, :].rearrange("p k q -> p (k q)"),
                    t_ones[:, s:s + ntile],
                    rai[:, s:s + ntile],
                    channels=128, num_elems=ntile * HI, num_idxs=ntile)
                s += ntile

            # ---- lo one-hots (DVE), pair-interleaved [kk, lo, k2] ----
            oh8p = wpool.tile([128, KT // 2, LO, 2], BF16, tag="oh8p")
            loo2 = loo.rearrange("p (kk two) -> p kk two", two=2)
            nc.vector.tensor_tensor(
                oh8p[:],
                c_i8row.unsqueeze(1).unsqueeze(3)
                    .broadcast_to((128, KT // 2, LO, 2)),
                loo2.unsqueeze(2).broadcast_to((128, KT // 2, LO, 2)),
                ALU.is_equal,
            )
            oh8t = wpool.tile([128, NT, LO], BF16, tag="oh8t")
            nc.vector.tensor_tensor(
                oh8t[:],
                c_i8row.unsqueeze(1).broadcast_to((128, NT, LO)),
                lot.unsqueeze(2).broadcast_to((128, NT, LO)),
                ALU.is_equal,
            )

            # ---- w2 moving operand chunks (DVE, pair layout for 2x) ----
            w2p = wpool.tile([128, KT // 2, LO, 9, 2], BF16, tag="w2p")
            for c0 in range(0, KT // 2, CH // 2):
                sl = slice(c0, c0 + CH // 2)
                nc.vector.tensor_tensor(
                    w2p[:, sl],
                    oh8p[:, sl].unsqueeze(3)
                        .broadcast_to((128, CH // 2, LO, 9, 2)),
                    t_ybfp[:, sl].unsqueeze(2)
                        .broadcast_to((128, CH // 2, LO, 9, 2)),
                    ALU.mult,
                )
            # ra tail on DVE
            nc.vector.tensor_tensor(
                rad[:],
                c_i128row.unsqueeze(1).broadcast_to((128, _RA_DVE, HI)),
                hio[:, KG:KT].unsqueeze(2).broadcast_to((128, _RA_DVE, HI)),
                ALU.is_equal,
            )

            # ---- pooling: 4 accumulating matmuls over w-phases ----
            # ycON is staged (c, w, y) so each phase's moving is contiguous
            yv = t_ycon.rearrange("p (c w y) -> p c w y", c=4, y=Y)
            pp = psP.tile([32, 32, Y], F32, tag="pp")
            for c in range(4):
                nc.tensor.matmul(pp[:], c_pmat, yv[:, c, :, :],
                                 start=(c == 0), stop=(c == 3))
            nc.scalar.copy(gvabf[:, :, 0:8], pp[:])
            # ypsb[p, m, :] == [gv(hi=p, lo=m, 0:8), 1.0]: cell s = 8p+m has
            # hi = p, lo = m exactly, so this is already (hi, lo) layout.
            ypsb = wpool.tile([128, 8, 9], BF16, tag="ypsb")
            nc.sync.dma_start(ypsb[:], gvabf[:])

            # ---- target transpose + broadcast + hi one-hot (ACT) ----
            pst = psP.tile([16, 128], F32, tag="pst")
            nc.tensor.transpose(pst[:], hit, c_ident)
            ihjTbf = wpool.tile([16, 128], BF16, tag="ihjTbf")
            nc.scalar.copy(ihjTbf[:], pst[:])

            rt4s = []
            for g in range(NG):
                pb4 = psB.tile([128, 4, 128], F32, tag="pb4")
                for m in range(4):
                    nc.tensor.matmul(pb4[:, m, :], c_sel[:, 4 * g + m, :],
                                     ihjTbf[:], start=True, stop=True)
                sq4 = wpool.tile([128, 4 * 128], F32, tag="sq4")
                nc.scalar.activation(sq4[:], pb4[:].rearrange("p m q -> p (m q)"),
                                     ACTF.Square, bias=c_niotaP, scale=1.0)
                rt4 = wpool.tile([128, 4, 128], BF16, tag=f"rt4_{g}")
                nc.scalar.activation(rt4[:].rearrange("p m q -> p (m q)"),
                                     sq4[:], ACTF.Relu, bias=1.0, scale=-1.0)
                rt4s.append(rt4)

            # ---- scatter matmul stream, ordered by producer readiness ----
            # dve-built tail tiles go before the last gpsimd chunk; the
            # on-grid identity matmul rides mid-stream, off the stop path
            ps = psS.tile([128, LO * 9], F32, tag="ps")
            order = [*range(0, KG - _RA_CALLS[-1]), *range(KG, KT),
                     *range(KG - _RA_CALLS[-1], KG)]
            half_pt = len(order) // 2
            for n, k in enumerate(order):
                if n == half_pt:
                    # fold in on-grid cell values (+1.0 count col)
                    nc.tensor.matmul(ps[:], c_identB,
                                     ypsb[:].rearrange("p m y -> p (m y)"),
                                     start=False, stop=False)
                rak = ra[:, k, :] if k < KG else rad[:, k - KG, :]
                nc.tensor.matmul(ps[:], rak,
                                 w2p[:, k // 2, :, :, k % 2],
                                 start=(n == 0), stop=(n == len(order) - 1))

            # ---- per-bin averages: avgM[128, (y, lo)] bf16 ----
            psv = ps[:].rearrange("p (l y) -> p l y", y=9)
            rc = wpool.tile([128, LO], F32, tag="rc")
            nc.vector.reciprocal(rc[:], psv[:, :, 8])
            avgM = wpool.tile([128, Y, LO], BF16, tag="avgM")
            nc.vector.tensor_tensor(
                avgM[:],
                psv[:, :, 0:8].transpose([0, 2, 1]),
                rc[:].unsqueeze(1).broadcast_to((128, Y, LO)),
                ALU.mult,
            )

            # ---- gather matmuls + lo contraction, two pipelined halves ----
            # both output DMAs ride the sync queue back-to-back so the
            # second's descriptor pipelines behind the first's ring-wake
            outsb = wpool.tile([128, NT, Y], BF16, tag="outsb")
            H = NT // 2
            odst = out_d[:].rearrange("(p n) y -> p (n y)", p=128)
            for h in range(2):
                rv = psR.tile([128, H, Y, LO], F32, tag=f"rv{h}")
                for j in range(H):
                    n = h * H + j
                    nc.tensor.matmul(
                        rv[:, j, :, :], rt4s[n // 4][:, n % 4, :],
                        avgM[:].rearrange("p y l -> p (y l)"),
                        start=True, stop=True)
                tmp = wpool.tile([128, H, Y, LO], BF16, tag=f"tmp{h}")
                nc.vector.tensor_tensor(
                    tmp[:],
                    rv[:],
                    oh8t[:, h * H:(h + 1) * H, :].unsqueeze(2)
                        .broadcast_to((128, H, Y, LO)),
                    ALU.mult,
                )
                # lo contraction as an all-bf16 packed add tree (2x mode)
                s1 = wpool.tile([128, H, Y, 4], BF16, tag=f"s1_{h}")
                nc.vector.tensor_tensor(s1[:], tmp[:, :, :, 0:4],
                                        tmp[:, :, :, 4:8], ALU.add)
                s2 = wpool.tile([128, H, Y, 2], BF16, tag=f"s2_{h}")
                nc.vector.tensor_tensor(s2[:], s1[:, :, :, 0:2],
                                        s1[:, :, :, 2:4], ALU.add)
                nc.vector.tensor_tensor(
                    outsb[:, h * H:(h + 1) * H, :].unsqueeze(3),
                    s2[:, :, :, 0:1], s2[:, :, :, 1:2], ALU.add)
                eng = nc.sync if h == 0 else nc.scalar
                eng.dma_start(odst[:, h * H * Y:(h + 1) * H * Y],
                              outsb[:, h * H:(h + 1) * H, :])
    nc.compile()
    return nc


def _consts():
    cE = np.zeros((128, _CE_COLS), np.float32)
    o = 0
    cE[:, o] = np.arange(128, dtype=np.float32); o += 1
    cE[:, o] = -np.arange(128, dtype=np.float32); o += 1
    cE[:, o:o + 8] = np.arange(8, dtype=np.float32)[None, :]; o += 8
    rabase = np.zeros(KT, np.float32)
    s = 0
    for ntile in _RA_CALLS:
        rabase[s:s + ntile] = 128.0 * np.arange(ntile)
        s += ntile
    assert s == KG
    cE[:, o:o + KT] = rabase[None, :]; o += KT
    cE[:, o:o + 128] = np.eye(128, dtype=np.float32); o += 128
    cE[:, o:o + 128] = np.arange(128, dtype=np.float32)[None, :]; o += 128
    assert o == _CE_COLS

    sel = (np.arange(16)[:, None] == np.arange(NT)[None, :])  # [16, NT]
    selb = np.repeat(sel[:, :, None], 128, axis=2).reshape(16, NT * 128)
    return {"constE": cE, "selB": selb.astype(ml_dtypes.bfloat16)}


def _stage_core(xc_off, yc_off, yc_on, xt, b, half):
    m = {}
    fin = np.empty((128, _IN_COLS), np.float32)
    sl = slice(half * TH, (half + 1) * TH)
    o = 0
    fin[:, o:o + KT] = xc_off[b, :, 0].reshape(KT, 128).T; o += KT
    # target (p, n) holds xt row p*16+n so the output DMA is contiguous
    fin[:, o:o + NT] = xt[b, sl, 0].reshape(128, NT); o += NT
    fin[:, o:o + KT] = xc_off[b, :, 1].reshape(KT, 128).T; o += KT
    fin[:, o:o + NT] = xt[b, sl, 1].reshape(128, NT); o += NT
    ones_bits = np.full((128, KT), 0x3f80, np.uint16)
    fin[:, o:o + KT // 2] = ones_bits.view(np.uint32).view(np.float32)
    o += KT // 2
    assert o == _IN_COLS
    m["inF"] = fin

    ybf = np.ones((128, KT, 9), np.float32)
    ybf[:, :, 0:8] = yc_off[b].reshape(KT, 128, Y).transpose(1, 0, 2)
    # pair-interleave: [p, kk, y, k2], k = 2*kk + k2
    ybfp = ybf.reshape(128, KT // 2, 2, 9).transpose(0, 1, 3, 2)
    m["ybf"] = np.ascontiguousarray(ybfp).reshape(
        128, KT * 9).astype(ml_dtypes.bfloat16)

    blob = np.zeros((128, _CB_COLS), np.float32)
    # ycON in (c, w, y) order: phase-c moving operand is contiguous
    yco = yc_on[b].reshape(128, 32, 4, Y).transpose(0, 2, 1, 3)
    blob[:, 0:1024] = yco.reshape(128, 1024)
    pmat = np.zeros((128, 32), np.float32)
    for h in range(128):
        pmat[h, h // 4] = 1.0 / 16.0
    blob[:, 1024:1056] = pmat
    blob[:, 1056:1184] = np.eye(128, dtype=np.float32)
    m["conB"] = blob.astype(ml_dtypes.bfloat16)
    return m


def _in_maps(inputs):
    xc_off_grid = np.ascontiguousarray(inputs["xc_off_grid"], np.float32)
    yc_off_grid = np.ascontiguousarray(inputs["yc_off_grid"], np.float32)
    yc_on_grid = np.ascontiguousarray(inputs["yc_on_grid"], np.float32)
    xt = np.ascontiguousarray(inputs["xt"], np.float32)
    consts = _consts()
    in_maps = []
    for core in range(8):
        b, half = core // 2, core % 2
        m = dict(consts)
        m.update(_stage_core(xc_off_grid, yc_off_grid, yc_on_grid, xt, b, half))
        in_maps.append(m)
    return in_maps


_NC = None


def kernel(xc_off_grid, yc_off_grid, xc_on_grid, yc_on_grid, xt):
    global _NC
    if _NC is None:
        _NC = build_nc()
    nc = _NC

    in_maps = _in_maps(dict(xc_off_grid=xc_off_grid, yc_off_grid=yc_off_grid,
                            yc_on_grid=yc_on_grid, xt=xt))

    res = run_bass_kernel_spmd(nc, in_maps, list(range(8)))
    out = np.empty((B, T, Y), np.float32)
    for core in range(8):
        b, half = core // 2, core % 2
        out[b, half * TH:(half + 1) * TH] = np.asarray(
            res.results[core]["out"]).astype(np.float32)
    return out
